# revision 2
# baseline (speedup 1.0000x reference)
"""AtomAttentionEncoder distributed across 8 trn2 NeuronCores.

Sharding: the 256 attention blocks are split 32-per-core (1024 atoms/core).
Each core gets its atoms plus a 192-atom halo on each side (1408 local
atoms), enough to run all 3 block-local attention layers without any
cross-core communication (the 128-wide key window grows the dependency
footprint by 48 atoms per side per layer; 192 covers 3 layers at block
granularity). Global-edge key-window clipping is handled exactly by
passing per-core key-window start indices as data and gathering on
device. Host-side numpy does only sharding / index prep / concatenation.
"""

import numpy as np
import jax
import jax.numpy as jnp
from functools import partial

B, N, A = 1, 8192, 4
T = N // A
C_ATOM, C_PAIR, C_TOKEN = 128, 16, 384
H, DH, L = 4, 32, 3
Q, K = 32, 128
NBLK = N // Q
ELEM, CHDIM = 128, 64

NDEV = 8
HALO = 192
NL = 1024 + 2 * HALO          # 1408 local atoms per core
JS, JE = 2, 42                # local block slots processed (40 blocks)
NBL = JE - JS
QS, QE = JS * Q, JE * Q       # query atoms [64, 1344) written each layer
C0, C1 = HALO, HALO + 1024    # central (owned) atom range
PC0, PC1 = 6 - JS, 38 - JS    # central blocks within processed array: [4, 36)


def _ln(x, eps=1e-5):
    m = jnp.mean(x, axis=-1, keepdims=True)
    v = jnp.var(x, axis=-1, keepdims=True)
    return (x - m) * jax.lax.rsqrt(v + eps)


def _fwd(positions, mask, element, charge, name_chars, uid, wuid, key_idx, p):
    """Per-core forward. positions [NL,3]; mask/charge [NL]; element/uid/wuid
    [NL] i32; name_chars [NL,4] i32; key_idx [NBL,K] i32 (local key rows per
    processed block, already globally clipped). Returns central shards."""
    f32 = jnp.float32
    m = mask[:, None]
    # ---- per_atom_cond ----
    elem_1h = jax.nn.one_hot(element, ELEM, dtype=f32)
    name_1h = jax.nn.one_hot(name_chars, CHDIM, dtype=f32).reshape(NL, 4 * CHDIM)
    act = (positions @ p['w_ref_pos'] + m @ p['w_ref_mask']
           + elem_1h @ p['w_ref_elem']
           + jnp.log(charge + jnp.sqrt(charge * charge + 1.0))[:, None]
           @ p['w_ref_charge']
           + name_1h @ p['w_ref_name'])
    single_cond = act * m                                   # [NL, C_ATOM]

    # ---- block-sparse pair activations [NBL, Q, K, C_PAIR] ----
    uid_q = uid[QS:QE].reshape(NBL, Q)[:, :, None]
    uid_r = wuid[key_idx][:, None, :]
    valid = (uid_q == uid_r)[..., None].astype(f32)
    pos_q = positions[QS:QE].reshape(NBL, Q, 1, 3)
    pos_r = positions[key_idx][:, None, :, :]
    offsets = pos_q - pos_r
    pair = (offsets @ p['w_off']) * valid
    sq_d = jnp.sum(offsets * offsets, axis=-1, keepdims=True)
    pair += ((1.0 / (1.0 + sq_d)) @ p['w_dist']) * valid
    rsc = jax.nn.relu(single_cond)
    row = rsc @ p['w_row']
    col = rsc @ p['w_col']
    pair += row[QS:QE].reshape(NBL, Q, C_PAIR)[:, :, None, :]
    pair += col[key_idx][:, None, :, :]
    pair += valid @ p['w_pmask']
    h = jax.nn.relu(pair) @ p['w_mlp1']
    h = jax.nn.relu(h) @ p['w_mlp2']
    h = jax.nn.relu(h) @ p['w_mlp3']
    pair = pair + h

    # ---- local-atom DiffusionTransformer ----
    a = single_cond
    s_ln = _ln(single_cond)
    s_ln_q = s_ln[QS:QE]
    pair_ln = _ln(pair)
    inv_sqrt_dh = np.float32(1.0 / np.sqrt(DH))
    for l in range(L):
        ln_a = _ln(a)
        aq = (jax.nn.sigmoid(s_ln_q @ p['wg_q'][l] + p['bg_q'][l]) * ln_a[QS:QE]
              + s_ln_q @ p['wb_q'][l])
        akv = (jax.nn.sigmoid(s_ln @ p['wg_kv'][l] + p['bg_kv'][l]) * ln_a
               + s_ln @ p['wb_kv'][l])
        q = jnp.einsum('nc,chd->nhd', aq, p['wq'][l]) + p['bq'][l]
        k = jnp.einsum('nc,chd->nhd', akv, p['wk'][l])
        v = jnp.einsum('nc,chd->nhd', akv, p['wv'][l])
        qb = q.reshape(NBL, Q, H, DH)
        kb = k[key_idx]
        vb = v[key_idx]
        logits = jnp.einsum('nqhd,nkhd->nhqk', qb, kb) * inv_sqrt_dh
        logits += jnp.einsum('nqkc,ch->nhqk', pair_ln, p['w_pb'][l])
        w = jax.nn.softmax(logits, axis=-1)
        o = jnp.einsum('nhqk,nkhd->nqhd', w, vb).reshape(NBL * Q, H * DH)
        g = jax.nn.sigmoid(
            jnp.einsum('nc,chd->nhd', aq, p['wg_attn'][l])).reshape(NBL * Q, H * DH)
        o = (g * o) @ p['wo'][l]
        b_attn = jax.nn.sigmoid(s_ln_q @ p['ws_gate'][l] - 2.0) * o
        at = (jax.nn.sigmoid(s_ln_q @ p['wg_t'][l] + p['bg_t'][l]) * ln_a[QS:QE]
              + s_ln_q @ p['wb_t'][l])
        hid = jax.nn.silu(at @ p['w_t1'][l]) * (at @ p['w_t2'][l])
        t = jax.nn.sigmoid(s_ln_q @ p['ws_gate_t'][l] - 2.0) * (hid @ p['w_t3'][l])
        a = a.at[QS:QE].set(b_attn + t)

    # ---- project to token level + masked mean over atoms per token ----
    a_c = a[C0:C1]
    tok = jax.nn.relu(a_c @ p['w_proj']).reshape(1024 // A, A, C_TOKEN)
    tmask = mask[C0:C1].reshape(1024 // A, A, 1)
    token_act = (jnp.sum(tok * tmask, axis=1)
                 / jnp.maximum(jnp.sum(tmask, axis=1), 1e-6))
    return token_act, a_c, single_cond[C0:C1], pair[PC0:PC1]


_pfwd = jax.pmap(_fwd, in_axes=(0, 0, 0, 0, 0, 0, 0, 0, None))


def kernel(positions, mask, element, charge, atom_name_chars, ref_space_uid,
           params):
    positions = np.asarray(positions)
    mask = np.asarray(mask)
    element = np.asarray(element)
    charge = np.asarray(charge)
    atom_name_chars = np.asarray(atom_name_chars)
    ref_space_uid = np.asarray(ref_space_uid)
    params = {k: np.asarray(v) for k, v in params.items()}

    # hotfix_mangle_layout: atoms in a token take the first atom's uid
    uid_g = ref_space_uid[0]
    wuid_g = np.broadcast_to(
        uid_g.reshape(T, A)[:, :1], (T, A)).reshape(N).astype(np.int32)

    # ---- shard: per-core local atom index maps + key-window indices ----
    idx_l, kidx_l = [], []
    for c in range(NDEV):
        start_c = 1024 * c - HALO
        idx = np.clip(start_c + np.arange(NL), 0, N - 1)
        g = 32 * c - 6 + np.arange(JS, JE)          # global block ids
        kstart = np.clip(g * Q + Q // 2 - K // 2, 0, N - K)
        koff = np.clip(kstart - start_c, 0, NL - K)
        kidx = koff[:, None] + np.arange(K)[None, :]
        idx_l.append(idx)
        kidx_l.append(kidx.astype(np.int32))
    idx_l = np.stack(idx_l)                          # [NDEV, NL]
    kidx_l = np.stack(kidx_l)                        # [NDEV, NBL, K]

    pos_l = positions[0][idx_l]                      # [NDEV, NL, 3]
    mask_l = mask[0][idx_l]
    elem_l = element[0][idx_l].astype(np.int32)
    charge_l = charge[0][idx_l]
    name_l = atom_name_chars[0][idx_l].astype(np.int32)
    uid_l = uid_g[idx_l].astype(np.int32)
    wuid_l = wuid_g[idx_l]

    token_act, a_c, sc_c, pair_c = _pfwd(
        pos_l, mask_l, elem_l, charge_l, name_l, uid_l, wuid_l, kidx_l, params)

    token_act = np.asarray(token_act).reshape(B, T, C_TOKEN)
    a_full = np.asarray(a_c).reshape(B, N, C_ATOM)
    sc_full = np.asarray(sc_c).reshape(B, N, C_ATOM)
    pair_full = np.asarray(pair_c).reshape(B, NBLK, Q, K, C_PAIR)
    return token_act, a_full, sc_full, pair_full


# revision 4
# speedup vs baseline: 32.8169x; 32.8169x over previous
"""AtomAttentionEncoder distributed across 8 trn2 NeuronCores.

Sharding: the 256 attention blocks are split 32-per-core (1024 atoms/core).
Each core gets its atoms plus a 192-atom halo on each side (1408 local
atoms), enough to run all 3 block-local attention layers without any
cross-core communication (the 128-wide key window grows the dependency
footprint by 48 atoms per side per layer; 192 covers 3 layers at block
granularity). Global-edge key-window clipping is handled exactly by
passing per-core key-window start indices as data and gathering on
device. Host-side numpy does only sharding / index prep / concatenation.
"""

import numpy as np
import jax
import jax.numpy as jnp
from functools import partial

B, N, A = 1, 8192, 4
T = N // A
C_ATOM, C_PAIR, C_TOKEN = 128, 16, 384
H, DH, L = 4, 32, 3
Q, K = 32, 128
NBLK = N // Q
ELEM, CHDIM = 128, 64

NDEV = 8
HALO = 192
NL = 1024 + 2 * HALO          # 1408 local atoms per core
JS, JE = 2, 42                # local block slots processed (40 blocks)
NBL = JE - JS
QS, QE = JS * Q, JE * Q       # query atoms [64, 1344) written each layer
C0, C1 = HALO, HALO + 1024    # central (owned) atom range
PC0, PC1 = 6 - JS, 38 - JS    # central blocks within processed array: [4, 36)


def _ln(x, eps=1e-5):
    m = jnp.mean(x, axis=-1, keepdims=True)
    v = jnp.var(x, axis=-1, keepdims=True)
    return (x - m) * jax.lax.rsqrt(v + eps)


def _fwd(positions, mask, element, charge, name_chars, uid, wuid, key_idx, p):
    """Per-core forward. positions [NL,3]; mask/charge [NL]; element/uid/wuid
    [NL] i32; name_chars [NL,4] i32; key_idx [NBL,K] i32 (local key rows per
    processed block, already globally clipped). Returns central shards."""
    f32 = jnp.float32
    m = mask[:, None]
    # ---- per_atom_cond ----
    elem_1h = jax.nn.one_hot(element, ELEM, dtype=f32)
    name_1h = jax.nn.one_hot(name_chars, CHDIM, dtype=f32).reshape(NL, 4 * CHDIM)
    act = (positions @ p['w_ref_pos'] + m @ p['w_ref_mask']
           + elem_1h @ p['w_ref_elem']
           + jnp.log(charge + jnp.sqrt(charge * charge + 1.0))[:, None]
           @ p['w_ref_charge']
           + name_1h @ p['w_ref_name'])
    single_cond = act * m                                   # [NL, C_ATOM]

    # ---- block-sparse pair activations [NBL, Q, K, C_PAIR] ----
    uid_q = uid[QS:QE].reshape(NBL, Q)[:, :, None]
    uid_r = wuid[key_idx][:, None, :]
    valid = (uid_q == uid_r)[..., None].astype(f32)
    pos_q = positions[QS:QE].reshape(NBL, Q, 1, 3)
    pos_r = positions[key_idx][:, None, :, :]
    offsets = pos_q - pos_r
    pair = (offsets @ p['w_off']) * valid
    sq_d = jnp.sum(offsets * offsets, axis=-1, keepdims=True)
    pair += ((1.0 / (1.0 + sq_d)) @ p['w_dist']) * valid
    rsc = jax.nn.relu(single_cond)
    row = rsc @ p['w_row']
    col = rsc @ p['w_col']
    pair += row[QS:QE].reshape(NBL, Q, C_PAIR)[:, :, None, :]
    pair += col[key_idx][:, None, :, :]
    pair += valid @ p['w_pmask']
    h = jax.nn.relu(pair) @ p['w_mlp1']
    h = jax.nn.relu(h) @ p['w_mlp2']
    h = jax.nn.relu(h) @ p['w_mlp3']
    pair = pair + h

    # ---- local-atom DiffusionTransformer ----
    a = single_cond
    s_ln = _ln(single_cond)
    s_ln_q = s_ln[QS:QE]
    pair_ln = _ln(pair)
    inv_sqrt_dh = np.float32(1.0 / np.sqrt(DH))
    for l in range(L):
        ln_a = _ln(a)
        aq = (jax.nn.sigmoid(s_ln_q @ p['wg_q'][l] + p['bg_q'][l]) * ln_a[QS:QE]
              + s_ln_q @ p['wb_q'][l])
        akv = (jax.nn.sigmoid(s_ln @ p['wg_kv'][l] + p['bg_kv'][l]) * ln_a
               + s_ln @ p['wb_kv'][l])
        q = jnp.einsum('nc,chd->nhd', aq, p['wq'][l]) + p['bq'][l]
        k = jnp.einsum('nc,chd->nhd', akv, p['wk'][l])
        v = jnp.einsum('nc,chd->nhd', akv, p['wv'][l])
        qb = q.reshape(NBL, Q, H, DH)
        kb = k[key_idx]
        vb = v[key_idx]
        logits = jnp.einsum('nqhd,nkhd->nhqk', qb, kb) * inv_sqrt_dh
        logits += jnp.einsum('nqkc,ch->nhqk', pair_ln, p['w_pb'][l])
        w = jax.nn.softmax(logits, axis=-1)
        o = jnp.einsum('nhqk,nkhd->nqhd', w, vb).reshape(NBL * Q, H * DH)
        g = jax.nn.sigmoid(
            jnp.einsum('nc,chd->nhd', aq, p['wg_attn'][l])).reshape(NBL * Q, H * DH)
        o = (g * o) @ p['wo'][l]
        b_attn = jax.nn.sigmoid(s_ln_q @ p['ws_gate'][l] - 2.0) * o
        at = (jax.nn.sigmoid(s_ln_q @ p['wg_t'][l] + p['bg_t'][l]) * ln_a[QS:QE]
              + s_ln_q @ p['wb_t'][l])
        hid = jax.nn.silu(at @ p['w_t1'][l]) * (at @ p['w_t2'][l])
        t = jax.nn.sigmoid(s_ln_q @ p['ws_gate_t'][l] - 2.0) * (hid @ p['w_t3'][l])
        a = a.at[QS:QE].set(b_attn + t)

    # ---- project to token level + masked mean over atoms per token ----
    a_c = a[C0:C1]
    tok = jax.nn.relu(a_c @ p['w_proj']).reshape(1024 // A, A, C_TOKEN)
    tmask = mask[C0:C1].reshape(1024 // A, A, 1)
    token_act = (jnp.sum(tok * tmask, axis=1)
                 / jnp.maximum(jnp.sum(tmask, axis=1), 1e-6))
    return token_act, a_c, single_cond[C0:C1], pair[PC0:PC1]


_pfwd = jax.pmap(_fwd, in_axes=(0, 0, 0, 0, 0, 0, 0, 0, 0))


def _shard_indices():
    """Per-core local atom index maps + key-window row indices (static)."""
    idx_l, kidx_l = [], []
    for c in range(NDEV):
        start_c = 1024 * c - HALO
        idx = np.clip(start_c + np.arange(NL), 0, N - 1)
        g = 32 * c - 6 + np.arange(JS, JE)          # global block ids
        kstart = np.clip(g * Q + Q // 2 - K // 2, 0, N - K)
        koff = np.clip(kstart - start_c, 0, NL - K)
        kidx = koff[:, None] + np.arange(K)[None, :]
        idx_l.append(idx)
        kidx_l.append(kidx.astype(np.int32))
    return np.stack(idx_l), np.stack(kidx_l)


_IDX, _KIDX = _shard_indices()                       # [NDEV,NL], [NDEV,NBL,K]
_PARAM_CACHE = {}


def _device_params(params):
    key = tuple(id(params[k]) for k in sorted(params))
    cached = _PARAM_CACHE.get(key)
    if cached is None:
        np_params = {k: np.asarray(v) for k, v in params.items()}
        cached = jax.device_put_replicated(np_params, jax.devices()[:NDEV])
        _PARAM_CACHE.clear()
        _PARAM_CACHE[key] = cached
    return cached


def kernel(positions, mask, element, charge, atom_name_chars, ref_space_uid,
           params):
    positions = np.asarray(positions)
    mask = np.asarray(mask)
    element = np.asarray(element)
    charge = np.asarray(charge)
    atom_name_chars = np.asarray(atom_name_chars)
    ref_space_uid = np.asarray(ref_space_uid)
    dparams = _device_params(params)

    # hotfix_mangle_layout: atoms in a token take the first atom's uid
    uid_g = ref_space_uid[0]
    wuid_g = np.broadcast_to(
        uid_g.reshape(T, A)[:, :1], (T, A)).reshape(N).astype(np.int32)
    idx_l, kidx_l = _IDX, _KIDX

    pos_l = positions[0][idx_l]                      # [NDEV, NL, 3]
    mask_l = mask[0][idx_l]
    elem_l = element[0][idx_l].astype(np.int32)
    charge_l = charge[0][idx_l]
    name_l = atom_name_chars[0][idx_l].astype(np.int32)
    uid_l = uid_g[idx_l].astype(np.int32)
    wuid_l = wuid_g[idx_l]

    token_act, a_c, sc_c, pair_c = _pfwd(
        pos_l, mask_l, elem_l, charge_l, name_l, uid_l, wuid_l, kidx_l,
        dparams)

    token_act = np.asarray(token_act).reshape(B, T, C_TOKEN)
    a_full = np.asarray(a_c).reshape(B, N, C_ATOM)
    sc_full = np.asarray(sc_c).reshape(B, N, C_ATOM)
    pair_full = np.asarray(pair_c).reshape(B, NBLK, Q, K, C_PAIR)
    return token_act, a_full, sc_full, pair_full
